# revision 5
# baseline (speedup 1.0000x reference)
"""BoxConv2d Trainium2 kernel.

Math: the reference computes, per output channel k = (c, f),
    out[b,k] = interp-row(I) diff  then  interp-col diff
where I is the zero-padded integral image of input[b,c].  The whole
pipeline (integral image + fractional box-edge interpolation) is linear
in the input and separable, so it collapses to

    out[b,k] = A_k @ x[b,c] @ B_k^T

with banded "pixel overlap" matrices
    A_k[xo, a] = clamp(xo + x_max_k + 1 - a, 0, 1) - clamp(xo + x_min_k - a, 0, 1)
    B_k[yo, j] = clamp(yo + y_max_k + 1 - j, 0, 1) - clamp(yo + y_min_k - j, 0, 1)

(A_k[xo, a] is exactly the length of the overlap between the box row
extent [xo + x_min, xo + x_max + 1] and the pixel row [a, a+1]; same for
columns.)  A/B are built on the host from the tiny (C,F) box params and
shipped per-core; the device does pure 128x128 matmuls on the PE array.

Sharding: output channels K = C*F = 128 are split across 8 cores
(16 channels = 4 in_planes per core).  Each core reads only its 4 input
planes, so input reads are not duplicated across the chip.
"""

import sys

if "/opt/trn_rl_repo" not in sys.path:
    sys.path.insert(0, "/opt/trn_rl_repo")

import numpy as np

import concourse.bass as bass  # noqa: F401
import concourse.mybir as mybir
import concourse.tile as tile
from concourse import bacc
from concourse.bass_utils import run_bass_kernel_spmd

B, C, F, H, W = 8, 32, 4, 128, 128
NCORES = 8
CPC = C // NCORES  # in_planes per core
KPC = CPC * F      # output channels per core

_DT = mybir.dt.float32

_NC_CACHE = {}
LAST_RESULT = None


def _build_nc():
    nc = bacc.Bacc(
        "TRN2", target_bir_lowering=False, debug=False, num_devices=NCORES
    )
    x_p = nc.declare_dram_parameter("x", [B, H, CPC * W], _DT, isOutput=False)
    at_p = nc.declare_dram_parameter("at", [H, KPC * H], _DT, isOutput=False)
    bt_p = nc.declare_dram_parameter("bt", [W, KPC * W], _DT, isOutput=False)
    out_p = nc.declare_dram_parameter("out", [B, H, KPC * W], _DT, isOutput=True)

    with tile.TileContext(nc) as tc:
        with (
            tc.tile_pool(name="const", bufs=1) as cpool,
            tc.tile_pool(name="xin", bufs=2) as xpool,
            tc.tile_pool(name="vsb", bufs=3) as vpool,
            tc.tile_pool(name="osb", bufs=3) as opool,
            tc.tile_pool(name="pv", bufs=2, space="PSUM") as pvpool,
            tc.tile_pool(name="po", bufs=2, space="PSUM") as popool,
        ):
            at_sb = cpool.tile([128, KPC * H], _DT, tag="at")
            nc.sync.dma_start(at_sb[:], at_p[:])
            bt_sb = cpool.tile([128, KPC * W], _DT, tag="bt")
            nc.sync.dma_start(bt_sb[:], bt_p[:])

            for b in range(B):
                x_sb = xpool.tile([128, CPC * W], _DT)
                nc.sync.dma_start(x_sb[:], x_p[b])
                for c in range(CPC):
                    # pass 1: V[j, (f,xo)] = sum_a x[a, j] * A_k[xo, a]
                    v_ps = pvpool.tile([128, F * H], mybir.dt.float32)
                    nc.tensor.matmul(
                        v_ps[:],
                        lhsT=x_sb[:, c * W:(c + 1) * W],
                        rhs=at_sb[:, c * F * H:(c + 1) * F * H],
                        start=True,
                        stop=True,
                    )
                    v_sb = vpool.tile([128, F * H], _DT)
                    nc.vector.tensor_copy(v_sb[:], v_ps[:])
                    # pass 2: out[xo, yo] = sum_j V[j, xo] * B_k[yo, j]
                    o_ps = popool.tile([128, F * W], mybir.dt.float32)
                    for f in range(F):
                        kl = c * F + f
                        nc.tensor.matmul(
                            o_ps[:, f * W:(f + 1) * W],
                            lhsT=v_sb[:, f * H:(f + 1) * H],
                            rhs=bt_sb[:, kl * W:(kl + 1) * W],
                            start=True,
                            stop=True,
                        )
                    o_sb = opool.tile([128, F * W], _DT)
                    nc.vector.tensor_copy(o_sb[:], o_ps[:])
                    nc.sync.dma_start(
                        out_p[b][:, c * F * W:(c + 1) * F * W], o_sb[:]
                    )
    nc.finalize()
    return nc


def _get_nc():
    if "nc" not in _NC_CACHE:
        _NC_CACHE["nc"] = _build_nc()
    return _NC_CACHE["nc"]


def _overlap_mats(lo, hi):
    """(K, out, in) pixel-overlap matrices for 128-wide axis."""
    t = np.arange(128, dtype=np.float64)
    d = t[:, None] - t[None, :]  # out - in
    lo = lo.astype(np.float64)[:, None, None]
    hi = hi.astype(np.float64)[:, None, None]
    m = np.clip(d[None] + hi + 1.0, 0.0, 1.0) - np.clip(d[None] + lo, 0.0, 1.0)
    return m.astype(np.float32)


def _make_in_maps(input, x_min, x_max, y_min, y_max):
    A = _overlap_mats(x_min.reshape(-1), x_max.reshape(-1))   # (K, xo, a)
    Bm = _overlap_mats(y_min.reshape(-1), y_max.reshape(-1))  # (K, yo, j)
    in_maps = []
    for m in range(NCORES):
        ks = slice(KPC * m, KPC * (m + 1))
        at = A[ks].transpose(2, 0, 1).reshape(H, KPC * H)     # [a, (kl, xo)]
        bt = Bm[ks].transpose(2, 0, 1).reshape(W, KPC * W)    # [j, (kl, yo)]
        xm = input[:, CPC * m:CPC * (m + 1)].transpose(0, 2, 1, 3)
        xm = xm.reshape(B, H, CPC * W)                        # [b, a, (c, j)]
        in_maps.append({
            "x": np.ascontiguousarray(xm, dtype=np.float32),
            "at": np.ascontiguousarray(at, dtype=np.float32),
            "bt": np.ascontiguousarray(bt, dtype=np.float32),
        })
    return in_maps


def _assemble(results):
    out = np.empty((B, C * F, H, W), np.float32)
    for m in range(NCORES):
        o = results[m]["out"].reshape(B, H, KPC, W).transpose(0, 2, 1, 3)
        out[:, KPC * m:KPC * (m + 1)] = o
    return out


def _run(inputs, trace=False):
    global LAST_RESULT
    nc = _get_nc()
    in_maps = _make_in_maps(**inputs)
    LAST_RESULT = run_bass_kernel_spmd(
        nc, in_maps, list(range(NCORES)), trace=trace
    )
    return _assemble(LAST_RESULT.results)


def kernel(input, x_min, x_max, y_min, y_max):
    return _run({
        "input": np.asarray(input),
        "x_min": np.asarray(x_min),
        "x_max": np.asarray(x_max),
        "y_min": np.asarray(y_min),
        "y_max": np.asarray(y_max),
    })
